# revision 4
# baseline (speedup 1.0000x reference)
"""Trainium2 Bass kernel for nn_AirFitMultiHeadDNN.

Model (per batch row b, head h):
    ev = emb[e[b,h]]                               (3)
    fv = f[b, 3h:3h+3] @ Wf + bf                   (5)
    x  = [ev, fv]                                  (8)
    h1 = leaky_relu(x @ W1[h] + b1[h], 0.01)       (10)
    z  = h1 @ W2[h] + b2[h]                        (1)
    out[b] = sum_h softplus(z)[h] * Wo[h] + bo

Kernel strategy (pure data parallel over 8 cores, batch-on-free layout):
  * Embedding lookup realized as matmuls with an exact exponential one-hot:
    Q[(h,j), b] = -L*(e[b,h]-j)^2 built by a matmul over host-supplied
    (e, e^2) feature rows; G = exp(Q) is exactly 1 at e==j and underflows
    to ~0 (L=100) otherwise.  j=0 is folded into the layer-1 bias via
    onehot_0 = 1 - sum_{j>=1} onehot_j, so G has 20*12=240 rows.
  * evT[60, n] = dEmbT @ G accumulated in PSUM; layer 1 contracts
    K=120 = [fT(60); evT(60)] against folded weights (Wf folded into W1).
  * leaky_relu decomposed exactly as leaky(t) = 0.01*t + 0.99*relu(t):
    the nonlinearity is a plain Relu pass (per-partition bias = b1),
    the W2 weighting lives in the z-matmul weights (0.99*W2), and the
    linear 0.01 term rides along as +/- zlin output-column pairs
    (t = relu(t) - relu(-t)) with weight +/-0.01.
  * z accumulated per head via static matmuls, column-tiled 4 chunks per
    PSUM bank; softplus(z+b2) = Ln(Exp(z+b2)+1) in two ScalarE passes
    over the stack (z in [-6, 3] so Exp is safe); final Wo contraction
    uses a [116, 4] mask matrix so 4 chunks finish in one matmul; +bo on
    the PSUM->SBUF evacuation.  Only Exp/Ln/Relu/Copy are used, all in
    the natural_log_exp_and_others activation table -> one table load.
"""

import numpy as np

import concourse.bacc as bacc
import concourse.tile as tile
from concourse import mybir
from concourse import bass_utils

F32 = mybir.dt.float32

B = 524288
H = 20
NUM_EX = 13
EMB = 3
FEAT_IN, FEAT_OUT = 3, 5
HEAD_IN, HEAD_HID = 8, 10

NCORES = 8
BC = B // NCORES

LBIG = 100.0
NJ = NUM_EX - 1          # j = 1..12 (j=0 folded into bias)
NG = H * NJ              # 240 exponential-one-hot rows
GC1 = 128                # G partition chunk sizes
GC2 = NG - GC1           # 112
KX = 60 + H * EMB        # 120 contraction rows for layer 1
RW = 120                 # layer-1 output rows per half: 100 relu + 2*10 zlin
CHUNK = 512              # batch columns per chunk
SUPER = 4                # chunks per z-stack super-iteration
SROWS = 3 * 32 + H       # 116 rows of the z stack that matter


def _np32(x):
    return np.ascontiguousarray(np.asarray(x), dtype=np.float32)


def build_consts(emb, Wf, bf, W1, b1, W2, b2, Wo, bo):
    """Host-side folding of the tiny parameter set into kernel constants."""
    emb = _np32(emb); Wf = _np32(Wf); bf = _np32(bf)
    W1 = _np32(W1); b1 = _np32(b1); W2 = _np32(W2); b2 = _np32(b2)
    Wo = _np32(Wo); bo = _np32(bo)

    # Q coefficients: Q[(h,j), b] = -L*e^2 + 2Lj*e - L*j^2  (bias holds -L*j^2)
    qcoef = np.zeros((40, NG), np.float32)
    ebias = np.zeros((NG, 1), np.float32)
    for h in range(H):
        for j in range(1, NUM_EX):
            r = h * NJ + (j - 1)
            qcoef[h, r] = 2.0 * LBIG * j
            qcoef[20 + h, r] = -LBIG
            ebias[r, 0] = -LBIG * j * j

    # Gather matrix: evT[3h+i, b] = sum_j G[(h,j), b] * (emb[j,i]-emb[0,i])
    membT = np.zeros((NG, 60), np.float32)
    for h in range(H):
        for j in range(1, NUM_EX):
            r = h * NJ + (j - 1)
            membT[r, 3 * h:3 * h + 3] = emb[j] - emb[0]

    # Layer-1 weights on K = [ev(60); f(60)], output cols = (h,o)
    # (ev rows first: the PSUM->SBUF copy must start at partition 0)
    w1all = np.zeros((KX, H * HEAD_HID), np.float32)
    b1eff = np.zeros(H * HEAD_HID, np.float32)
    for h in range(H):
        WfW1 = Wf @ W1[h, FEAT_IN:, :]            # (3, 10)
        for o in range(HEAD_HID):
            c = h * HEAD_HID + o
            w1all[3 * h:3 * h + 3, c] = W1[h, :EMB, o]
            w1all[60 + 3 * h:60 + 3 * h + 3, c] = WfW1[:, o]
            b1eff[c] = (b1[h, o] + bf @ W1[h, FEAT_IN:, o]
                        + emb[0] @ W1[h, :EMB, o])

    # Augmented layer-1: per half 100 relu rows + 10 (+zlin) + 10 (-zlin),
    # where zlin[h] = sum_o W2[h,o] * (pre_raw + b1)[h,o].
    w2flat = W2.reshape(H * HEAD_HID)
    w1aug = np.zeros((KX, 2 * RW), np.float32)
    rbias = np.zeros((2 * RW, 1), np.float32)
    zw = np.zeros((2, RW, 32), np.float32)
    for hf in range(2):
        for hh in range(10):
            h = hf * 10 + hh
            zlin_w = np.zeros(KX, np.float32)
            zlin_b = 0.0
            for o in range(HEAD_HID):
                c = h * HEAD_HID + o
                r = hf * RW + hh * HEAD_HID + o
                w1aug[:, r] = w1all[:, c]
                rbias[r, 0] = b1eff[c]
                zw[hf, hh * HEAD_HID + o, h] = 0.99 * w2flat[c]
                zlin_w += w2flat[c] * w1all[:, c]
                zlin_b += w2flat[c] * b1eff[c]
            rp = hf * RW + 100 + hh
            rn = hf * RW + 110 + hh
            w1aug[:, rp] = zlin_w
            w1aug[:, rn] = -zlin_w
            rbias[rp, 0] = zlin_b
            rbias[rn, 0] = -zlin_b
            zw[hf, 100 + hh, h] = 0.01
            zw[hf, 110 + hh, h] = -0.01

    # Softplus bias (z stack rows 32c+h get b2[h]) and final Wo mask
    b2rep = np.zeros((SROWS, 1), np.float32)
    wo4 = np.zeros((SROWS, 4), np.float32)
    for c in range(SUPER):
        for h in range(H):
            r = 32 * c + h
            if r < SROWS:
                b2rep[r, 0] = b2[h]
                wo4[r, c] = Wo[h, 0]

    bo4 = np.full((4, 1), float(bo[0]), np.float32)

    return {
        "qcoef1": np.ascontiguousarray(qcoef[:, :GC1]),
        "qcoef2": np.ascontiguousarray(qcoef[:, GC1:]),
        "ebias1": np.ascontiguousarray(ebias[:GC1]),
        "ebias2": np.ascontiguousarray(ebias[GC1:]),
        "membT1": np.ascontiguousarray(membT[:GC1]),
        "membT2": np.ascontiguousarray(membT[GC1:]),
        "w1aug": w1aug,
        "rbias1": np.ascontiguousarray(rbias[:RW]),
        "rbias2": np.ascontiguousarray(rbias[RW:]),
        "zw1": np.ascontiguousarray(zw[0]),
        "zw2": np.ascontiguousarray(zw[1]),
        "b2rep": b2rep,
        "wo4": wo4,
        "bo4": bo4,
    }


CONST_SHAPES = {
    "qcoef1": (40, GC1), "qcoef2": (40, GC2),
    "ebias1": (GC1, 1), "ebias2": (GC2, 1),
    "membT1": (GC1, 60), "membT2": (GC2, 60),
    "w1aug": (KX, 2 * RW),
    "rbias1": (RW, 1), "rbias2": (RW, 1),
    "zw1": (RW, 32), "zw2": (RW, 32),
    "b2rep": (SROWS, 1), "wo4": (SROWS, 4),
    "bo4": (4, 1),
}


def build_program(bc):
    """Build the per-core Bass program for a batch shard of bc rows."""
    assert bc % (CHUNK * SUPER) == 0
    nsuper = bc // (CHUNK * SUPER)

    nc = bacc.Bacc("TRN2", target_bir_lowering=False, debug=False)

    ft_d = nc.dram_tensor("ft", (60, bc), F32, kind="ExternalInput")
    eq_d = nc.dram_tensor("eq", (40, bc), F32, kind="ExternalInput")
    out_d = nc.dram_tensor("out", (bc,), F32, kind="ExternalOutput")
    const_d = {
        name: nc.dram_tensor(name, shape, F32, kind="ExternalInput")
        for name, shape in CONST_SHAPES.items()
    }

    EXP = mybir.ActivationFunctionType.Exp
    RELU = mybir.ActivationFunctionType.Relu
    LN = mybir.ActivationFunctionType.Ln

    with tile.TileContext(nc) as tc:
        with (
            tc.tile_pool(name="consts", bufs=1) as consts,
            tc.tile_pool(name="xcat", bufs=3) as xcat_pool,
            tc.tile_pool(name="eq", bufs=3) as eq_pool,
            tc.tile_pool(name="g", bufs=4) as g_pool,
            tc.tile_pool(name="h1", bufs=4) as h1_pool,
            tc.tile_pool(name="sp", bufs=2) as s_pool,
            tc.tile_pool(name="ot", bufs=2) as out_pool,
            tc.tile_pool(name="ps_q", bufs=1, space="PSUM") as ps_q,
            tc.tile_pool(name="ps_ev", bufs=2, space="PSUM") as ps_ev,
            tc.tile_pool(name="ps_pre", bufs=2, space="PSUM") as ps_pre,
            tc.tile_pool(name="ps_z", bufs=2, space="PSUM") as ps_z,
            tc.tile_pool(name="ps_o", bufs=1, space="PSUM") as ps_o,
        ):
            cs = {}
            for name, shape in CONST_SHAPES.items():
                t = consts.tile(list(shape), F32, tag=name)
                nc.sync.dma_start(out=t[:], in_=const_d[name][:])
                cs[name] = t

            qcoefs = [cs["qcoef1"], cs["qcoef2"]]
            ebiases = [cs["ebias1"], cs["ebias2"]]
            membTs = [cs["membT1"], cs["membT2"]]
            gsizes = [GC1, GC2]
            rbiases = [cs["rbias1"], cs["rbias2"]]
            zws = [cs["zw1"], cs["zw2"]]

            for s in range(nsuper):
                z_ps = ps_z.tile([128, CHUNK], F32, tag="z")
                for ci in range(SUPER):
                    col0 = (s * SUPER + ci) * CHUNK
                    xcat = xcat_pool.tile([KX, CHUNK], F32, tag="xcat")
                    nc.sync.dma_start(
                        out=xcat[60:KX, :], in_=ft_d[:, col0:col0 + CHUNK])
                    eq = eq_pool.tile([40, CHUNK], F32, tag="eq")
                    nc.sync.dma_start(
                        out=eq[:], in_=eq_d[:, col0:col0 + CHUNK])

                    # Exponential one-hot: G = exp(-L*(e-j)^2), exact 0/1.
                    gs = []
                    for qc in range(2):
                        m = gsizes[qc]
                        q_ps = ps_q.tile([128, CHUNK], F32, tag="q")
                        nc.tensor.matmul(
                            q_ps[:m, :], qcoefs[qc][:, :], eq[:],
                            start=True, stop=True)
                        g = g_pool.tile([128, CHUNK], F32, tag="g")
                        nc.scalar.activation(
                            g[:m, :], q_ps[:m, :], EXP,
                            bias=ebiases[qc][:, :], scale=1.0)
                        gs.append(g)

                    # Gather: evT = dEmbT @ G, accumulated over both chunks.
                    ev_ps = ps_ev.tile([60, CHUNK], F32, tag="ev")
                    nc.tensor.matmul(
                        ev_ps[:], membTs[0][:, :], gs[0][:GC1, :],
                        start=True, stop=False)
                    nc.tensor.matmul(
                        ev_ps[:], membTs[1][:, :], gs[1][:GC2, :],
                        start=False, stop=True)
                    nc.vector.tensor_copy(xcat[0:60, :], ev_ps[:])

                    # Layer 1 (+zlin columns), Relu, and per-head z sums into
                    # the column-tiled z stack.
                    for hf in range(2):
                        pre_ps = ps_pre.tile([RW, CHUNK], F32, tag="pre")
                        nc.tensor.matmul(
                            pre_ps[:],
                            cs["w1aug"][:, hf * RW:(hf + 1) * RW],
                            xcat[:], start=True, stop=True)
                        h1 = h1_pool.tile([RW, CHUNK], F32, tag="h1")
                        nc.scalar.activation(
                            h1[:], pre_ps[:], RELU,
                            bias=rbiases[hf][:, :], scale=1.0)
                        nc.tensor.matmul(
                            z_ps[32 * ci:32 * ci + 32, :],
                            zws[hf][:, :], h1[:],
                            start=(hf == 0), stop=(hf == 1),
                            tile_position=(0, 32 * ci))

                # softplus(z + b2) = Ln(Exp(z + b2) + 1) over the 4-chunk
                # stack, then the masked Wo contraction gives all 4 chunk
                # outputs in one matmul.
                ez = s_pool.tile([SROWS, CHUNK], F32, tag="ez")
                nc.scalar.activation(
                    ez[:], z_ps[:SROWS, :], EXP,
                    bias=cs["b2rep"][:, :], scale=1.0)
                sT = s_pool.tile([SROWS, CHUNK], F32, tag="sT")
                nc.scalar.activation(sT[:], ez[:], LN, bias=1.0, scale=1.0)
                o_ps = ps_o.tile([4, CHUNK], F32, tag="o")
                nc.tensor.matmul(
                    o_ps[:], cs["wo4"][:, :], sT[:], start=True, stop=True)
                outt = out_pool.tile([4, CHUNK], F32, tag="outt")
                nc.vector.tensor_scalar_add(outt[:], o_ps[:], cs["bo4"][:, :])
                ov = out_d[s * SUPER * CHUNK:(s + 1) * SUPER * CHUNK]
                nc.sync.dma_start(
                    out=ov.rearrange("(c n) -> c n", c=4), in_=outt[:])

    nc.compile()
    return nc


_PROGRAM_CACHE = {}


def _get_program(bc):
    if bc not in _PROGRAM_CACHE:
        _PROGRAM_CACHE[bc] = build_program(bc)
    return _PROGRAM_CACHE[bc]


def make_in_maps(e, f, consts, ncores=NCORES):
    e = np.asarray(e)
    f = np.asarray(f)
    btot = e.shape[0]
    bc = btot // ncores
    ef = e.astype(np.float32)
    e2f = (e.astype(np.int64) ** 2).astype(np.float32)
    in_maps = []
    for c in range(ncores):
        sl = slice(c * bc, (c + 1) * bc)
        ft = np.ascontiguousarray(f[sl].T.astype(np.float32))
        eq = np.ascontiguousarray(
            np.concatenate([ef[sl].T, e2f[sl].T], axis=0))
        m = {"ft": ft, "eq": eq}
        m.update(consts)
        in_maps.append(m)
    return in_maps


def kernel(e, f, emb, Wf, bf, W1, b1, W2, b2, Wo, bo, **kwargs):
    consts = build_consts(emb, Wf, bf, W1, b1, W2, b2, Wo, bo)
    nc = _get_program(BC)
    in_maps = make_in_maps(e, f, consts)
    res = bass_utils.run_bass_kernel_spmd(
        nc, in_maps, core_ids=list(range(NCORES)))
    out = np.concatenate([res.results[c]["out"] for c in range(NCORES)])
    return out.reshape(-1, 1).astype(np.float32)
